# revision 3
# baseline (speedup 1.0000x reference)
"""Performer (FAVOR+) attention block on 8 Trainium2 NeuronCores.

Math (per batch b):
    kp  = exp(k @ w.T - |k|^2/2) / sqrt(m)        [T, m]
    qp  = exp(q @ w.T - |q|^2/2) / sqrt(m)        [T, m]
    D   = qp @ (kp.sum(axis=0))                   [T, 1]
    kptv = v.T @ kp                               [d, m]
    y   = (qp @ kptv.T) / (D + eps)               [T, d]
    out = y @ proj_w.T                            [T, d]

We fold proj into the small matrix:  C' = (proj_w @ kptv).T  [m, d], so
    out = (qp @ C') / (D + eps)
which removes the [T,d]x[d,d] projection matmul (4x the flops of C').

Sharding: 8 cores = 4 batches x 2 query-halves. Each core computes the
k-side (kp, kptv, ksum) over the full T=4096 keys of its batch and the
q-side for its half (2048 queries). Matmul operands run in bf16 (cast
in-flight by DMA), accumulation in fp32 PSUM.
"""

import math

import numpy as np

import concourse.bass as bass
import concourse.mybir as mybir
import concourse.tile as tile
from concourse import bacc, bass_utils
from concourse.masks import make_identity

F32 = mybir.dt.float32
BF16 = mybir.dt.bfloat16
AF = mybir.ActivationFunctionType

N_CORES = 8
B, T, D_MODEL, M = 4, 4096, 1024, 512
TQ = T // 2                       # queries per core
DT = D_MODEL // 128               # 8 d tiles
MT = M // 128                     # 4 m tiles
RK = T // 128                     # 32 key token tiles
RQ = TQ // 128                    # 16 query token tiles
NEG_HALF_LOG_M = -0.5 * math.log(M)
EPS = 1e-8


def _build_program():
    nc = bacc.Bacc("TRN2", target_bir_lowering=False, debug=False,
                   num_devices=N_CORES)

    kT_d = nc.dram_tensor("kT", [D_MODEL, T], F32, kind="ExternalInput")
    v_d = nc.dram_tensor("v", [T, D_MODEL], F32, kind="ExternalInput")
    qT_d = nc.dram_tensor("qT", [D_MODEL, TQ], F32, kind="ExternalInput")
    wT_d = nc.dram_tensor("wT", [D_MODEL, M], F32, kind="ExternalInput")
    pwT_d = nc.dram_tensor("pwT", [D_MODEL, D_MODEL], F32, kind="ExternalInput")
    out_d = nc.dram_tensor("out", [TQ, D_MODEL], F32, kind="ExternalOutput")

    with tile.TileContext(nc) as tc:
        with (
            tc.tile_pool(name="res", bufs=1) as res,
            tc.tile_pool(name="xstream", bufs=16) as xstream,
            tc.tile_pool(name="vstream", bufs=4) as vstream,
            tc.tile_pool(name="small", bufs=8) as small,
            tc.tile_pool(name="outp", bufs=3) as outp,
        ):
            # ---- resident SBUF tensors ----
            wT_b = res.tile([128, DT * M], BF16, tag="wT_b")
            pwT_b = res.tile([128, DT * D_MODEL], BF16, tag="pwT_b")
            kp_sb = res.tile([128, RK * M], BF16, tag="kp_sb")
            qpT_sb = res.tile([128, MT * TQ], BF16, tag="qpT_sb")
            kptvT_sb = res.tile([128, MT * D_MODEL], BF16, tag="kptvT_sb")
            kptv_sb = res.tile([128, DT * M], BF16, tag="kptv_sb")
            C_sb = res.tile([128, MT * D_MODEL], BF16, tag="C_sb")
            ksum_sb = res.tile([128, MT], BF16, tag="ksum_sb")
            ident = res.tile([128, 128], BF16, tag="ident")
            ones_col = res.tile([128, 1], BF16, tag="ones_col")
            neghalf_col = res.tile([128, 1], BF16, tag="neghalf_col")

            make_identity(nc, ident[:])
            nc.gpsimd.memset(ones_col[:], 1.0)
            nc.gpsimd.memset(neghalf_col[:], -0.5)
            for dt in range(DT):
                nc.gpsimd.dma_start(
                    wT_b[:, dt * M:(dt + 1) * M],
                    wT_d[dt * 128:(dt + 1) * 128, :])
                nc.gpsimd.dma_start(
                    pwT_b[:, dt * D_MODEL:(dt + 1) * D_MODEL],
                    pwT_d[dt * 128:(dt + 1) * 128, :])

            # ---- phases K and Q: wtx -> exp -> kp / qpT ----
            def prm_phase(src_d, n_tok_tiles, psum_mm, psum_xd, psum_tr):
                """Compute prm_exp for src (kT or qT layout [d, Ntok])."""
                n_chunks = n_tok_tiles // 4
                for cc in range(n_chunks):
                    xt = [xstream.tile([128, 512], BF16, tag="xt", name=f"xt{dt}")
                          for dt in range(DT)]
                    sq = [xstream.tile([128, 512], BF16, tag="sq", name=f"sq{dt}")
                          for dt in range(DT)]
                    for dt in range(DT):
                        nc.gpsimd.dma_start(
                            xt[dt][:],
                            src_d[dt * 128:(dt + 1) * 128,
                                  cc * 512:(cc + 1) * 512])
                        nc.vector.tensor_mul(sq[dt][:], xt[dt][:], xt[dt][:])
                    for rl in range(4):
                        r = cc * 4 + rl
                        wtx = psum_mm.tile([128, M], F32, tag="wtx")
                        xd = psum_xd.tile([128, 1], F32, tag="xd")
                        for dt in range(DT):
                            nc.tensor.matmul(
                                wtx[:],
                                xt[dt][:, rl * 128:(rl + 1) * 128],
                                wT_b[:, dt * M:(dt + 1) * M],
                                start=(dt == 0), stop=(dt == DT - 1))
                        for dt in range(DT):
                            nc.tensor.matmul(
                                xd[:],
                                sq[dt][:, rl * 128:(rl + 1) * 128],
                                neghalf_col[:],
                                start=(dt == 0), stop=(dt == DT - 1))
                        bias = small.tile([128, 1], F32, tag="bias")
                        nc.scalar.activation(bias[:], xd[:], AF.Copy,
                                             bias=NEG_HALF_LOG_M)
                        yield r, wtx, bias

            with (
                tc.tile_pool(name="psum_mm", bufs=2,
                             space=bass.MemorySpace.PSUM) as psum_mm,
                tc.tile_pool(name="psum_xd", bufs=2,
                             space=bass.MemorySpace.PSUM) as psum_xd,
                tc.tile_pool(name="psum_tr", bufs=2,
                             space=bass.MemorySpace.PSUM) as psum_tr,
            ):
                for r, wtx, bias in prm_phase(kT_d, RK, psum_mm, psum_xd,
                                              psum_tr):
                    nc.scalar.activation(kp_sb[:, r * M:(r + 1) * M], wtx[:],
                                         AF.Exp, bias=bias[:])
                for r, wtx, bias in prm_phase(qT_d, RQ, psum_mm, psum_xd,
                                              psum_tr):
                    qp = xstream.tile([128, M], BF16, tag="qp")
                    nc.scalar.activation(qp[:], wtx[:], AF.Exp, bias=bias[:])
                    for mt in range(MT):
                        tr = psum_tr.tile([128, 128], BF16, tag="tr")
                        nc.tensor.transpose(
                            tr[:], qp[:, mt * 128:(mt + 1) * 128], ident[:])
                        nc.vector.tensor_copy(
                            qpT_sb[:, mt * TQ + r * 128: mt * TQ + (r + 1) * 128],
                            tr[:])

            # ---- phase KPTV: kptv^T[m, d] accumulated over all keys ----
            with tc.tile_pool(name="psum_kptv", bufs=1,
                              space=bass.MemorySpace.PSUM) as psum_kptv:
                pk = [psum_kptv.tile([128, D_MODEL], F32, tag=f"pk{mt}", name=f"pk{mt}")
                      for mt in range(MT)]
                for r in range(RK):
                    vt = vstream.tile([128, D_MODEL], BF16, tag="vt")
                    nc.gpsimd.dma_start(vt[:], v_d[r * 128:(r + 1) * 128, :])
                    for mt in range(MT):
                        lhs = kp_sb[:, r * M + mt * 128: r * M + (mt + 1) * 128]
                        for half in range(2):
                            nc.tensor.matmul(
                                pk[mt][:, half * 512:(half + 1) * 512],
                                lhs, vt[:, half * 512:(half + 1) * 512],
                                start=(r == 0), stop=(r == RK - 1))
                for mt in range(MT):
                    nc.scalar.activation(
                        kptvT_sb[:, mt * D_MODEL:(mt + 1) * D_MODEL],
                        pk[mt][:], AF.Copy)

            # ---- phase KSUM + transpose kptv^T -> kptv ----
            with (
                tc.tile_pool(name="psum_ks", bufs=1,
                             space=bass.MemorySpace.PSUM) as psum_ks,
                tc.tile_pool(name="psum_tr2", bufs=4,
                             space=bass.MemorySpace.PSUM) as psum_tr2,
            ):
                ks = psum_ks.tile([128, MT], F32, tag="ks")
                for mt in range(MT):
                    for r in range(RK):
                        nc.tensor.matmul(
                            ks[:, mt:mt + 1],
                            kp_sb[:, r * M + mt * 128: r * M + (mt + 1) * 128],
                            ones_col[:],
                            start=(r == 0), stop=(r == RK - 1))
                nc.scalar.activation(ksum_sb[:], ks[:], AF.Copy)

                for mt in range(MT):
                    for dt in range(DT):
                        tr = psum_tr2.tile([128, 128], BF16, tag="tr2")
                        nc.tensor.transpose(
                            tr[:],
                            kptvT_sb[:, mt * D_MODEL + dt * 128:
                                     mt * D_MODEL + (dt + 1) * 128],
                            ident[:])
                        nc.vector.tensor_copy(
                            kptv_sb[:, dt * M + mt * 128: dt * M + (mt + 1) * 128],
                            tr[:])

            # ---- phase C: C' = (proj_w @ kptv).T  [m, dout] ----
            with tc.tile_pool(name="psum_C", bufs=2,
                              space=bass.MemorySpace.PSUM) as psum_C:
                for mt in range(MT):
                    pc = psum_C.tile([128, D_MODEL], F32, tag="pc")
                    for dt in range(DT):
                        lhs = kptv_sb[:, dt * M + mt * 128: dt * M + (mt + 1) * 128]
                        for half in range(2):
                            nc.tensor.matmul(
                                pc[:, half * 512:(half + 1) * 512],
                                lhs,
                                pwT_b[:, dt * D_MODEL + half * 512:
                                      dt * D_MODEL + (half + 1) * 512],
                                start=(dt == 0), stop=(dt == DT - 1))
                    nc.scalar.activation(
                        C_sb[:, mt * D_MODEL:(mt + 1) * D_MODEL],
                        pc[:], AF.Copy)

            # ---- phase OUT: out = (qp @ C') / (D + eps) ----
            with (
                tc.tile_pool(name="psum_o", bufs=2,
                             space=bass.MemorySpace.PSUM) as psum_o,
                tc.tile_pool(name="psum_D", bufs=2,
                             space=bass.MemorySpace.PSUM) as psum_D,
            ):
                for r in range(RQ):
                    po = [psum_o.tile([128, 512], F32, tag=f"po{h}", name=f"po{h}")
                          for h in range(2)]
                    pD = psum_D.tile([128, 1], F32, tag="pD")
                    for mt in range(MT):
                        lhs = qpT_sb[:, mt * TQ + r * 128: mt * TQ + (r + 1) * 128]
                        for half in range(2):
                            nc.tensor.matmul(
                                po[half][:], lhs,
                                C_sb[:, mt * D_MODEL + half * 512:
                                     mt * D_MODEL + (half + 1) * 512],
                                start=(mt == 0), stop=(mt == MT - 1))
                        nc.tensor.matmul(pD[:], lhs, ksum_sb[:, mt:mt + 1],
                                         start=(mt == 0), stop=(mt == MT - 1))
                    Dp = small.tile([128, 1], F32, tag="Dp")
                    recD = small.tile([128, 1], F32, tag="recD")
                    nc.scalar.activation(Dp[:], pD[:], AF.Copy, bias=EPS)
                    nc.vector.reciprocal(recD[:], Dp[:])
                    ot = outp.tile([128, D_MODEL], F32, tag="ot")
                    for half in range(2):
                        nc.vector.tensor_scalar_mul(
                            ot[:, half * 512:(half + 1) * 512],
                            po[half][:], recD[:])
                    nc.sync.dma_start(out_d[r * 128:(r + 1) * 128, :], ot[:])

    nc.compile()
    return nc


_NC_CACHE = None


def _get_program():
    global _NC_CACHE
    if _NC_CACHE is None:
        _NC_CACHE = _build_program()
    return _NC_CACHE


def _make_in_maps(q, k, v, w, proj_w):
    wT = np.ascontiguousarray(w.T)
    pwT = np.ascontiguousarray(proj_w.T)
    in_maps = []
    for c in range(N_CORES):
        b, h = divmod(c, 2)
        in_maps.append({
            "kT": np.ascontiguousarray(k[b].T),
            "v": np.ascontiguousarray(v[b]),
            "qT": np.ascontiguousarray(q[b, h * TQ:(h + 1) * TQ].T),
            "wT": wT,
            "pwT": pwT,
        })
    return in_maps


def run(q, k, v, w, proj_w, trace=False, tmpdir=None):
    nc = _get_program()
    in_maps = _make_in_maps(q, k, v, w, proj_w)
    res = bass_utils.run_bass_kernel_spmd(
        nc, in_maps, core_ids=list(range(N_CORES)), trace=trace,
        tmpdir=tmpdir)
    out = np.empty((B, T, D_MODEL), dtype=np.float32)
    for c in range(N_CORES):
        b, h = divmod(c, 2)
        out[b, h * TQ:(h + 1) * TQ] = res.results[c]["out"]
    return out, res


def kernel(q, k, v, w, proj_w):
    out, _ = run(np.asarray(q, dtype=np.float32),
                 np.asarray(k, dtype=np.float32),
                 np.asarray(v, dtype=np.float32),
                 np.asarray(w, dtype=np.float32),
                 np.asarray(proj_w, dtype=np.float32))
    return out


# revision 6
# speedup vs baseline: 1.1689x; 1.1689x over previous
"""Performer (FAVOR+) attention block on 8 Trainium2 NeuronCores.

Math (per batch b):
    kp  = exp(k @ w.T - |k|^2/2) / sqrt(m)        [T, m]
    qp  = exp(q @ w.T - |q|^2/2) / sqrt(m)        [T, m]
    D   = qp @ (kp.sum(axis=0))                   [T, 1]
    kptv = v.T @ kp                               [d, m]
    y   = (qp @ kptv.T) / (D + eps)               [T, d]
    out = y @ proj_w.T                            [T, d]

Folds: out = (qp @ C') / (D + eps) with C' = (proj_w @ kptv).T [m, d],
removing the [T,d]x[d,d] projection matmul.

Sharding: 8 cores = 4 batches x 2 token-halves. Each core computes the
k-side (kp, kptv partial, ksum partial) for ITS half of the keys, then a
pairwise AllReduce (cores 2b, 2b+1) sums kptv/ksum; the q-side and output
are computed for the core's own query half. Matmul operands are bf16
(cast in-flight by DMA), accumulation fp32 in PSUM.

Layouts (SBUF [partition, free]):
  kt/qt  [d128 x 8dt, Tc]     token chunk of k/q, transposed on host
  wtxT   [m128, T512] psum    via wT-stationary matmuls (LDW reuse x4)
  xdT    [1, Tc]              -0.5*|x|^2 row, via (-1/2)-column matmuls
  kp_sb  [T128, m] bf16       via PE transpose of exp(wtxT)
  qpT_sb [m128, mt*TQ] bf16   direct exp output (no transpose needed)
  kptvT  [m128, d] psum/sbuf  kp-stationary matmuls, rhs v N=256 (LDW x4)
  C_sb   [m128, dout]         kptv-stationary matmuls over pwT
  out    [T128, dout]         qpT-stationary matmuls over C_sb + D column
"""

import math

import numpy as np

import concourse.bass as bass
import concourse.mybir as mybir
import concourse.tile as tile
from concourse import bacc, bass_utils
from concourse.masks import make_identity

F32 = mybir.dt.float32
BF16 = mybir.dt.bfloat16
AF = mybir.ActivationFunctionType

N_CORES = 8
B, T, D_MODEL, M = 4, 4096, 1024, 512
TC = T // 2                       # tokens per core (keys AND queries)
DT = D_MODEL // 128               # 8 d tiles
MT = M // 128                     # 4 m tiles
RC = TC // 128                    # 16 token tiles per core
NCH = TC // 512                   # 4 512-token chunks per core
NEG_HALF_LOG_M = -0.5 * math.log(M)
EPS = 1e-8
CC_GROUPS = [[0, 1], [2, 3], [4, 5], [6, 7]]


def _build_program():
    nc = bacc.Bacc("TRN2", target_bir_lowering=False, debug=False,
                   num_devices=N_CORES)

    kT_d = nc.dram_tensor("kT", [D_MODEL, TC], F32, kind="ExternalInput")
    v_d = nc.dram_tensor("v", [TC, D_MODEL], F32, kind="ExternalInput")
    qT_d = nc.dram_tensor("qT", [D_MODEL, TC], F32, kind="ExternalInput")
    wT_d = nc.dram_tensor("wT", [D_MODEL, M], F32, kind="ExternalInput")
    pwT_d = nc.dram_tensor("pwT", [D_MODEL, D_MODEL], F32, kind="ExternalInput")
    out_d = nc.dram_tensor("out", [TC, D_MODEL], F32, kind="ExternalOutput")

    with tile.TileContext(nc) as tc:
        with (
            tc.tile_pool(name="res", bufs=1) as res,
            tc.tile_pool(name="xstream", bufs=12) as xstream,
            tc.tile_pool(name="sqstream", bufs=2) as sqstream,
            tc.tile_pool(name="vstream", bufs=4) as vstream,
            tc.tile_pool(name="small", bufs=8) as small,
            tc.tile_pool(name="outp", bufs=3) as outp,
            tc.tile_pool(name="dram", bufs=1, space="DRAM") as dram,
        ):
            # ---- resident SBUF tensors ----
            wT_b = res.tile([128, DT * M], BF16, tag="wT_b")
            pwT_b = res.tile([128, DT * D_MODEL], BF16, tag="pwT_b")
            kp_sb = res.tile([128, RC * M], BF16, tag="kp_sb")
            qpT_sb = res.tile([128, MT * TC], BF16, tag="qpT_sb")
            kptvT_sb = res.tile([128, MT * D_MODEL], BF16, tag="kptvT_sb")
            kptv_sb = res.tile([128, DT * M], BF16, tag="kptv_sb")
            C_sb = res.tile([128, MT * D_MODEL], BF16, tag="C_sb")
            ksum_sb = res.tile([128, MT], BF16, tag="ksum_sb")
            xdT_k = res.tile([1, TC], BF16, tag="xdT_k")
            xdT_q = res.tile([1, TC], BF16, tag="xdT_q")
            ident = res.tile([128, 128], BF16, tag="ident")
            ones_col = res.tile([128, 1], BF16, tag="ones_col")
            ones_row = res.tile([1, 128], BF16, tag="ones_row")
            neghalf_col = res.tile([128, 1], BF16, tag="neghalf_col")
            expbias = res.tile([128, 1], F32, tag="expbias")

            cc_in = dram.tile([128, MT * D_MODEL + MT], BF16, tag="cc_in")
            cc_out = dram.tile([128, MT * D_MODEL + MT], BF16, tag="cc_out")

            make_identity(nc, ident[:])
            nc.gpsimd.memset(ones_col[:], 1.0)
            nc.gpsimd.memset(ones_row[:], 1.0)
            nc.gpsimd.memset(neghalf_col[:], -0.5)
            nc.gpsimd.memset(expbias[:], NEG_HALF_LOG_M)
            for dt in range(DT):
                nc.gpsimd.dma_start(
                    wT_b[:, dt * M:(dt + 1) * M],
                    wT_d[dt * 128:(dt + 1) * 128, :])

            def load_xt(src_d):
                """DMA-cast a [d, TC] operand into 8 bf16 [128, TC] tiles."""
                xt = []
                for dt in range(DT):
                    t = xstream.tile([128, TC], BF16, tag="xt",
                                     name=f"xt{dt}")
                    nc.gpsimd.dma_start(
                        t[:], src_d[dt * 128:(dt + 1) * 128, :])
                    xt.append(t)
                return xt

            def xd_phase(xt, xdT_out, psum_xd):
                """xdT_out[0, t] = -0.5 * sum_d x[d, t]^2  (bf16 row)."""
                xd = [psum_xd.tile([1, 512], F32, tag="xdp", name=f"xdp{c}")
                      for c in range(NCH)]
                for dt in range(DT):
                    sq = sqstream.tile([128, TC], BF16, tag="sq")
                    nc.vector.tensor_mul(sq[:], xt[dt][:], xt[dt][:])
                    for c in range(NCH):
                        nc.tensor.matmul(
                            xd[c][:], neghalf_col[:],
                            sq[:, c * 512:(c + 1) * 512],
                            start=(dt == 0), stop=(dt == DT - 1))
                for c in range(NCH):
                    nc.scalar.activation(
                        xdT_out[0:1, c * 512:(c + 1) * 512], xd[c][:],
                        AF.Copy)

            def wtx_phase(xt, xdT, psum_wtx, emit):
                """wtxT = w @ x - xd per m-tile; emit(mt, c, psum)."""
                for mt in range(MT):
                    ps = [psum_wtx.tile([128, 512], F32, tag="wtx",
                                        name=f"wtx{c}")
                          for c in range(NCH)]
                    for dt in range(DT):
                        lhs = wT_b[:, dt * M + mt * 128: dt * M + (mt + 1) * 128]
                        for c in range(NCH):
                            nc.tensor.matmul(
                                ps[c][:], lhs,
                                xt[dt][:, c * 512:(c + 1) * 512],
                                start=(dt == 0), stop=False)
                    for c in range(NCH):
                        nc.tensor.matmul(
                            ps[c][:], ones_row[:],
                            xdT[0:1, c * 512:(c + 1) * 512],
                            start=False, stop=True)
                        emit(mt, c, ps[c])

            # ================= K side =================
            kt = load_xt(kT_d)
            with tc.tile_pool(name="psum_xdk", bufs=4,
                              space=bass.MemorySpace.PSUM) as psum_xd:
                xd_phase(kt, xdT_k, psum_xd)

            with (
                tc.tile_pool(name="psum_wtxk", bufs=6,
                             space=bass.MemorySpace.PSUM) as psum_wtx,
                tc.tile_pool(name="psum_trk", bufs=2,
                             space=bass.MemorySpace.PSUM) as psum_tr,
            ):
                def emit_k(mt, c, ps):
                    kpT = small.tile([128, 512], BF16, tag="kpT")
                    nc.scalar.activation(kpT[:], ps[:], AF.Exp,
                                         bias=expbias[:])
                    for sub in range(4):
                        r = c * 4 + sub
                        tr = psum_tr.tile([128, 128], BF16, tag="trk")
                        nc.tensor.transpose(
                            tr[:], kpT[:, sub * 128:(sub + 1) * 128],
                            ident[:])
                        nc.vector.tensor_copy(
                            kp_sb[:, r * M + mt * 128: r * M + (mt + 1) * 128],
                            tr[:])
                wtx_phase(kt, xdT_k, psum_wtx, emit_k)

            # ---- KPTV partial + KSUM partial ----
            with (
                tc.tile_pool(name="psum_kptv", bufs=1,
                             space=bass.MemorySpace.PSUM) as psum_kptv,
            ):
                pk = [psum_kptv.tile([128, D_MODEL], F32, tag=f"pk{mt}",
                                     name=f"pk{mt}")
                      for mt in range(MT)]
                for r in range(RC):
                    vt = vstream.tile([128, D_MODEL], BF16, tag="vt")
                    nc.gpsimd.dma_start(vt[:], v_d[r * 128:(r + 1) * 128, :])
                    for mt in range(MT):
                        lhs = kp_sb[:, r * M + mt * 128: r * M + (mt + 1) * 128]
                        for qtr in range(4):
                            nc.tensor.matmul(
                                pk[mt][:, qtr * 256:(qtr + 1) * 256],
                                lhs, vt[:, qtr * 256:(qtr + 1) * 256],
                                start=(r == 0), stop=(r == RC - 1))
                for mt in range(MT):
                    st = outp.tile([128, D_MODEL], BF16, tag="ccst",
                                   name="ccst")
                    nc.scalar.activation(st[:], pk[mt][:], AF.Copy)
                    nc.sync.dma_start(
                        cc_in[:, mt * D_MODEL:(mt + 1) * D_MODEL], st[:])

            with tc.tile_pool(name="psum_ks", bufs=1,
                              space=bass.MemorySpace.PSUM) as psum_ks:
                ks = psum_ks.tile([1, M], F32, tag="ks")
                for r in range(RC):
                    nc.tensor.matmul(ks[:], ones_col[:],
                                     kp_sb[:, r * M:(r + 1) * M],
                                     start=(r == 0), stop=(r == RC - 1))
                ksr = small.tile([1, M], F32, tag="ksr")
                nc.scalar.activation(ksr[:], ks[:], AF.Copy)
                # scatter [1, 512] -> [128, 4]: cc_in[p, mt] = ksum[mt*128+p]
                nc.gpsimd.dma_start(
                    cc_in[:, MT * D_MODEL: MT * D_MODEL + MT],
                    ksr[0:1, :].rearrange("a (mt p) -> p (mt a)", p=128))

            # q-side loads issued before the AllReduce occupies gpsimd
            qt = load_xt(qT_d)

            # ---- pairwise AllReduce of kptv^T + ksum ----
            nc.gpsimd.collective_compute(
                "AllReduce", mybir.AluOpType.add, replica_groups=CC_GROUPS,
                ins=[cc_in.opt()], outs=[cc_out.opt()])
            nc.sync.dma_start(kptvT_sb[:], cc_out[:, 0: MT * D_MODEL])
            nc.sync.dma_start(ksum_sb[:],
                              cc_out[:, MT * D_MODEL: MT * D_MODEL + MT])

            # ================= Q side (overlaps the AllReduce) ============
            with tc.tile_pool(name="psum_xdq", bufs=4,
                              space=bass.MemorySpace.PSUM) as psum_xd:
                xd_phase(qt, xdT_q, psum_xd)
            with tc.tile_pool(name="psum_wtxq", bufs=6,
                              space=bass.MemorySpace.PSUM) as psum_wtx:
                def emit_q(mt, c, ps):
                    nc.scalar.activation(
                        qpT_sb[:, mt * TC + c * 512: mt * TC + (c + 1) * 512],
                        ps[:], AF.Exp, bias=expbias[:])
                wtx_phase(qt, xdT_q, psum_wtx, emit_q)

            # ---- load pwT late (only needed for C) ----
            for dt in range(DT):
                nc.gpsimd.dma_start(
                    pwT_b[:, dt * D_MODEL:(dt + 1) * D_MODEL],
                    pwT_d[dt * 128:(dt + 1) * 128, :])

            # ---- transpose kptv^T -> kptv ----
            with tc.tile_pool(name="psum_tr2", bufs=4,
                              space=bass.MemorySpace.PSUM) as psum_tr2:
                for mt in range(MT):
                    for dt in range(DT):
                        tr = psum_tr2.tile([128, 128], BF16, tag="tr2")
                        nc.tensor.transpose(
                            tr[:],
                            kptvT_sb[:, mt * D_MODEL + dt * 128:
                                     mt * D_MODEL + (dt + 1) * 128],
                            ident[:])
                        nc.vector.tensor_copy(
                            kptv_sb[:, dt * M + mt * 128: dt * M + (mt + 1) * 128],
                            tr[:])

            # ---- C' = (proj_w @ kptv).T [m, dout] ----
            with tc.tile_pool(name="psum_C", bufs=2,
                              space=bass.MemorySpace.PSUM) as psum_C:
                for mt in range(MT):
                    pc = psum_C.tile([128, D_MODEL], F32, tag="pc")
                    for dt in range(DT):
                        lhs = kptv_sb[:, dt * M + mt * 128: dt * M + (mt + 1) * 128]
                        for qtr in range(4):
                            nc.tensor.matmul(
                                pc[:, qtr * 256:(qtr + 1) * 256],
                                lhs,
                                pwT_b[:, dt * D_MODEL + qtr * 256:
                                      dt * D_MODEL + (qtr + 1) * 256],
                                start=(dt == 0), stop=(dt == DT - 1))
                    nc.scalar.activation(
                        C_sb[:, mt * D_MODEL:(mt + 1) * D_MODEL],
                        pc[:], AF.Copy)

            # ---- OUT: out = (qp @ C') / (D + eps) ----
            with (
                tc.tile_pool(name="psum_o", bufs=2,
                             space=bass.MemorySpace.PSUM) as psum_o,
                tc.tile_pool(name="psum_D", bufs=2,
                             space=bass.MemorySpace.PSUM) as psum_D,
            ):
                for r in range(RC):
                    po = psum_o.tile([128, D_MODEL], F32, tag="po")
                    pD = psum_D.tile([128, 1], F32, tag="pD")
                    for mt in range(MT):
                        lhs = qpT_sb[:, mt * TC + r * 128: mt * TC + (r + 1) * 128]
                        for qtr in range(4):
                            nc.tensor.matmul(
                                po[:, qtr * 256:(qtr + 1) * 256], lhs,
                                C_sb[:, mt * D_MODEL + qtr * 256:
                                     mt * D_MODEL + (qtr + 1) * 256],
                                start=(mt == 0), stop=(mt == MT - 1))
                        nc.tensor.matmul(pD[:], lhs, ksum_sb[:, mt:mt + 1],
                                         start=(mt == 0), stop=(mt == MT - 1))
                    Dp = small.tile([128, 1], F32, tag="Dp")
                    recD = small.tile([128, 1], F32, tag="recD")
                    nc.scalar.activation(Dp[:], pD[:], AF.Copy, bias=EPS)
                    nc.vector.reciprocal(recD[:], Dp[:])
                    ot = outp.tile([128, D_MODEL], F32, tag="ot")
                    for half in range(2):
                        nc.vector.tensor_scalar_mul(
                            ot[:, half * 512:(half + 1) * 512],
                            po[:, half * 512:(half + 1) * 512], recD[:])
                    nc.sync.dma_start(out_d[r * 128:(r + 1) * 128, :], ot[:])

    nc.compile()
    return nc


_NC_CACHE = None


def _get_program():
    global _NC_CACHE
    if _NC_CACHE is None:
        _NC_CACHE = _build_program()
    return _NC_CACHE


def _make_in_maps(q, k, v, w, proj_w):
    wT = np.ascontiguousarray(w.T)
    pwT = np.ascontiguousarray(proj_w.T)
    in_maps = []
    for c in range(N_CORES):
        b, h = divmod(c, 2)
        sl = slice(h * TC, (h + 1) * TC)
        in_maps.append({
            "kT": np.ascontiguousarray(k[b, sl].T),
            "v": np.ascontiguousarray(v[b, sl]),
            "qT": np.ascontiguousarray(q[b, sl].T),
            "wT": wT,
            "pwT": pwT,
        })
    return in_maps


def run(q, k, v, w, proj_w, trace=False, tmpdir=None):
    nc = _get_program()
    in_maps = _make_in_maps(q, k, v, w, proj_w)
    res = bass_utils.run_bass_kernel_spmd(
        nc, in_maps, core_ids=list(range(N_CORES)), trace=trace,
        tmpdir=tmpdir)
    out = np.empty((B, T, D_MODEL), dtype=np.float32)
    for c in range(N_CORES):
        b, h = divmod(c, 2)
        out[b, h * TC:(h + 1) * TC] = res.results[c]["out"]
    return out, res


def kernel(q, k, v, w, proj_w):
    out, _ = run(np.asarray(q, dtype=np.float32),
                 np.asarray(k, dtype=np.float32),
                 np.asarray(v, dtype=np.float32),
                 np.asarray(w, dtype=np.float32),
                 np.asarray(proj_w, dtype=np.float32))
    return out
